# revision 38
# baseline (speedup 1.0000x reference)
"""Trainium2 Bass kernel for GNN message passing (nn_BDLModule_34488587387542).

Computation (N=100000 nodes, E=1600000 edges, DIM=128):
    deg  = out-degree(src);  a = rsqrt(deg)
    h0   = per-node block rotation of x (8 bundles of 4x4)
    h2   = S S h0,  S = diag(a) A^T diag(a)   (2 propagation steps)
    h3   = inverse rotation of h2
    out  = GELU_exact(h3 @ w1.T + b1) @ w2.T + b2

Sharding: nodes partitioned contiguously across 8 cores (12500 each). Edges
bucketed by owning dst shard; per core, grouped by (dst tile of 128, src
chunk of 25088 replica rows) so gathers use int16 indices, and sorted by
src row within each bucket so the dma_gather descriptors walk ascending
HBM addresses (big win: random 256B row reads are row-miss bound). The
propagation step is: dma_gather rows from the replicated node table; the
one-hot dst matrices are PRECOMPUTED ON THE HOST as fp8e4 (exact 0/1) and
streamed from HBM (onehot_mode="host_fp8"; "tt"/"ts" build them on DVE);
PE matmuls (fp8 stationary x fp16 moving) accumulate into PSUM. AllGather
replicates the node table between steps (measured ~8us each - cheap). The
separable norm coefs a[src]*a[dst] are folded into the stored node tables;
the R1-side `a` is folded into x on the host. All rotation math runs in
fp16 on DVE (batched per supertile); FFN matmuls run in bf16.

build_nc(pipeline_reps=N) chains N independent copies of the whole pipeline
in one NEFF (slope timing tool; kernel() uses N=1).
"""
import os
import sys

sys.path.append("/opt/trn_rl_repo")

import numpy as np

N_NODES = 100000
N_EDGES = 1600000
DIM = 128
HID = 256
N_CORES = 8
NSH = 12500                 # nodes per shard
NSHP = 12544                # padded shard rows (98 * 128)
NT = NSHP // 128            # dst tiles per core = 98
NREP = NSHP * N_CORES       # replica table rows = 100352
CHUNK = NREP // 4           # gather chunk rows = 25088 (int16-addressable)
N_CHUNKS = 4
PAD_DST = 1000.0            # dst_local sentinel for padding edges
TS = 7                      # dst tiles per super-tile (gathers merged per
NS = NT // TS               # (super-tile, chunk) to amortize SWDGE overhead)

# module globals: last run state (test.py reuses these for timing)
LAST_RESULTS = None
LAST_NC = None
LAST_IN_MAPS = None


# ----------------------------------------------------------------- host prep

def _wrap_idx(idx_flat: np.ndarray) -> np.ndarray:
    """[n] -> [128, n/16] int16 wrapped+replicated dma_gather index layout."""
    w = idx_flat.reshape(-1, 16).T.astype(np.int16)
    return np.tile(w, (8, 1))


def preprocess(x, node_rep, src, dst, w1, b1, w2, b2):
    """Build per-core input maps + the static SPMD edge-group structure."""
    deg = np.bincount(src, minlength=N_NODES).astype(np.float64)
    a64 = 1.0 / np.sqrt(deg)
    a = a64.astype(np.float32)
    a2 = (1.0 / deg).astype(np.float32)

    # global node id -> replica-table row
    def rrow(u):
        return (u // NSH) * NSHP + (u % NSH)

    src_rrow = rrow(src)
    dst_core = dst // NSH

    # Buckets ordered (super-tile s, chunk k, tile-within-s): one gather per
    # (s, k) covers TS tiles' groups contiguously.
    n_buckets = NT * N_CHUNKS
    counts = np.zeros((N_CORES, n_buckets), np.int64)
    per_core = []
    for c in range(N_CORES):
        m = dst_core == c
        dl = (dst[m] - c * NSH).astype(np.int64)      # local dst
        sr = src_rrow[m]
        tile_id = dl // 128
        chunk_id = sr // CHUNK
        key = ((tile_id // TS) * N_CHUNKS + chunk_id) * TS + tile_id % TS
        # secondary sort by src row: ascending gather addresses per bucket
        order = np.argsort(key * (1 << 18) + sr, kind="stable")
        per_core.append((dl[order], sr[order], key[order]))
        counts[c] = np.bincount(key, minlength=n_buckets)

    G = np.ceil(counts.max(axis=0) / 128.0).astype(np.int64)  # [n_buckets]
    g_off = np.concatenate([[0], np.cumsum(G)])               # group offsets
    total_groups = int(g_off[-1])
    total_edges_padded = total_groups * 128

    # tile-major views for the build loops
    Gmat = np.zeros((NT, N_CHUNKS), np.int64)
    g_start = np.zeros((NT, N_CHUNKS), np.int64)
    for b in range(n_buckets):
        s, k, u = b // (N_CHUNKS * TS), (b // TS) % N_CHUNKS, b % TS
        t = s * TS + u
        Gmat[t, k] = G[b]
        g_start[t, k] = g_off[b]
    structure = {"Gmat": Gmat, "g_start": g_start}

    iota = np.tile(np.arange(128, dtype=np.float16), (128, 1))
    ident = np.eye(128, dtype=np.float32)
    w1t = np.ascontiguousarray(w1.T.astype(np.float32))        # [DIM, HID]
    b1h = np.ascontiguousarray(b1.reshape(2, 128).T)           # [128, 2]
    w2t2 = np.ascontiguousarray(
        w2.T.reshape(2, 128, DIM).transpose(1, 0, 2).astype(np.float32))
    b2c = np.ascontiguousarray(b2.reshape(128, 1))

    in_maps = []
    for c in range(N_CORES):
        dl, sr, key = per_core[c]
        n = dl.shape[0]
        # position of each edge inside the padded stream
        bucket_starts = g_off[:-1] * 128                    # [n_buckets]
        within = np.arange(n) - np.concatenate(
            [[0], np.cumsum(np.bincount(key, minlength=n_buckets))]
        )[key]
        pos = bucket_starts[key] + within
        idx_pad = np.zeros(total_edges_padded, np.int64)
        dst_pad = np.full(total_edges_padded, PAD_DST, np.float16)
        idx_pad[pos] = sr % CHUNK
        dst_pad[pos] = (dl % 128).astype(np.float16)

        rows = slice(c * NSH, (c + 1) * NSH)
        # R1-side `a` folded into x here (rotation is linear per node)
        x_sh = np.zeros((NSHP, DIM), np.float16)
        x_sh[:NSH] = (x[rows] * a[rows, None]).astype(np.float16)
        rep_sh = np.zeros((NSHP, DIM), np.float16)
        rep_sh[:NSH] = node_rep[rows].reshape(NSH, DIM).astype(np.float16)
        a_pad = np.zeros(NSHP, np.float32)
        a_pad[:NSH] = a[rows]
        a2_pad = np.zeros(NSHP, np.float32)
        a2_pad[:NSH] = a2[rows]

        # host-built one-hot matrices, fp8e4 (exact 0/1): [128, tg, 128]
        # m2_host[p, g, f] = 1 iff edge slot p of group g has dst row f
        # (0x38 is the e4m3 bit pattern of 1.0 - avoids slow ml_dtypes cast)
        from ml_dtypes import float8_e4m3
        dst_g = dst_pad.reshape(total_groups, 128).astype(np.int32)  # [g, p]
        m2_host = np.zeros((128, total_groups, 128), np.uint8)
        gi, pi = np.nonzero(dst_g < 128)
        m2_host[pi, gi, dst_g[gi, pi]] = 0x38
        m2_host = m2_host.view(float8_e4m3)

        in_maps.append({
            "x_sh": x_sh,
            "rep_sh": rep_sh,
            "idx_all": _wrap_idx(idx_pad),                       # [128, tg*8]
            "dst_all": np.ascontiguousarray(
                dst_pad.reshape(total_groups, 128).T),           # [128, tg] f16
            "m2_all": m2_host,                                   # [128, tg, 128]
            "a_col": np.ascontiguousarray(
                a_pad.reshape(NT, 128).T),                       # [128, NT]
            "a2_col": np.ascontiguousarray(
                a2_pad.reshape(NT, 128).T),                      # [128, NT]
            "iota": iota,
            "ident": ident,
            "w1t": w1t,
            "b1h": b1h,
            "w2t2": w2t2,
            "b2c": b2c,
        })
    return in_maps, structure, total_groups


# -------------------------------------------------------------- device build

def build_nc(structure, total_groups, single_core_timing=False, ablate=(),
             n_queues=4, onehot_mode="host_fp8", pipeline_reps=1):
    import concourse.bacc as bacc
    import concourse.mybir as mybir
    import concourse.tile as tile

    f32 = mybir.dt.float32
    f16 = mybir.dt.float16
    bf16 = mybir.dt.bfloat16
    f8 = mybir.dt.float8e4
    nc = bacc.Bacc("TRN2", target_bir_lowering=False, debug=False,
                   num_devices=1 if single_core_timing else N_CORES,
                   num_swdge_queues=n_queues)

    x_sh = nc.dram_tensor("x_sh", [NSHP, DIM], f16, kind="ExternalInput")
    rep_sh = nc.dram_tensor("rep_sh", [NSHP, DIM], f16, kind="ExternalInput")
    idx_all = nc.dram_tensor("idx_all", [128, total_groups * 8],
                             mybir.dt.int16, kind="ExternalInput")
    dst_all = nc.dram_tensor("dst_all", [128, total_groups], f16,
                             kind="ExternalInput")
    m2_all = nc.dram_tensor("m2_all", [128, total_groups, 128], f8,
                            kind="ExternalInput")
    a_col = nc.dram_tensor("a_col", [128, NT], f32, kind="ExternalInput")
    a2_col = nc.dram_tensor("a2_col", [128, NT], f32, kind="ExternalInput")
    iota = nc.dram_tensor("iota", [128, 128], f16, kind="ExternalInput")
    ident = nc.dram_tensor("ident", [128, 128], f32, kind="ExternalInput")
    w1t = nc.dram_tensor("w1t", [DIM, HID], f32, kind="ExternalInput")
    b1h = nc.dram_tensor("b1h", [128, 2], f32, kind="ExternalInput")
    w2t2 = nc.dram_tensor("w2t2", [128, 2, DIM], f32, kind="ExternalInput")
    b2c = nc.dram_tensor("b2c", [128, 1], f32, kind="ExternalInput")
    out_t = nc.dram_tensor("out_t", [DIM, NSHP], f32, kind="ExternalOutput")

    Gmat = structure["Gmat"]
    g_start = structure["g_start"]
    # widest merged gather (groups) over (super-tile, chunk)
    gsk = Gmat.reshape(NS, TS, N_CHUNKS).sum(axis=1)        # [NS, N_CHUNKS]
    gsk_max = int(gsk.max())
    # widest one-hot batch (groups) over (tile, chunk) buckets
    g_tk_max = int(Gmat.max())
    # widest whole-super-tile group range (all 4 chunks, contiguous)
    gst = gsk.sum(axis=1)                                   # [NS]
    gst_max = int(gst.max())

    with tile.TileContext(nc) as tc:
        with (
            tc.tile_pool(name="const", bufs=1) as cp,
            tc.tile_pool(name="io", bufs=3) as iop,
            tc.tile_pool(name="rotp", bufs=2) as rotp,
            tc.tile_pool(name="gath", bufs=8) as gp,
            tc.tile_pool(name="m2", bufs=2 if onehot_mode == "host_fp8"
                         else 8) as m2p,
            tc.tile_pool(name="outp", bufs=3) as op,
            tc.tile_pool(name="psum", bufs=2, space="PSUM") as pp,
            tc.tile_pool(name="dram", bufs=1, space="DRAM") as dp,
        ):
            # ---- constants into SBUF
            iota_sb = cp.tile([128, 128], f16)
            nc.sync.dma_start(iota_sb[:], iota[:])
            id_sb = cp.tile([128, 128], f32)
            nc.sync.dma_start(id_sb[:], ident[:])
            idx_sb = cp.tile([128, total_groups * 8], mybir.dt.int16)
            nc.sync.dma_start(idx_sb[:], idx_all[:])
            if onehot_mode != "host_fp8":
                dst_sb = cp.tile([128, total_groups], f16)
                nc.sync.dma_start(dst_sb[:], dst_all[:])
            if onehot_mode == "ts":
                dst32_sb = cp.tile([128, total_groups], f32)
                nc.vector.tensor_copy(dst32_sb[:], dst_sb[:])
            a_sb = cp.tile([128, NT], f32)
            nc.sync.dma_start(a_sb[:], a_col[:])
            a2_sb = cp.tile([128, NT], f32)
            nc.sync.dma_start(a2_sb[:], a2_col[:])
            w1t_sb = cp.tile([DIM, HID], bf16)
            w1t_f32 = cp.tile([DIM, HID], f32)
            nc.sync.dma_start(w1t_f32[:], w1t[:])
            nc.vector.tensor_copy(w1t_sb[:], w1t_f32[:])
            b1h_sb = cp.tile([128, 2], f32)
            nc.sync.dma_start(b1h_sb[:], b1h[:])
            w2t2_sb = cp.tile([128, 2, DIM], bf16)
            w2t2_f32 = cp.tile([128, 2, DIM], f32)
            nc.sync.dma_start(w2t2_f32[:], w2t2[:])
            nc.vector.tensor_copy(w2t2_sb[:], w2t2_f32[:])
            b2c_sb = cp.tile([128, 1], f32)
            nc.sync.dma_start(b2c_sb[:], b2c[:])

            rep_space = "Local" if single_core_timing else "Shared"

            def allgather(sh, rep):
                if single_core_timing:
                    # timing stand-in: local copy keeps the data dependency
                    nc.sync.dma_start(rep[0:NSHP, :], sh[:])
                else:
                    nc.gpsimd.collective_compute(
                        "AllGather", mybir.AluOpType.bypass,
                        ins=[sh.opt()], outs=[rep.opt()],
                        replica_groups=[list(range(N_CORES))],
                    )

            def rotation_st(x_ap, rep_ap, dest, dest_tag, transposed):
                """Whole-super-tile rotation: dest = einsum(rep, x) per node.

                x_ap/rep_ap/dest: [128, TS, DIM] fp16. The (u, b) dims merge
                (stride(u)=128=8*16=8*stride(b)) so free rank stays <= 3.
                """
                UB = TS * 8
                x4 = x_ap.rearrange("p u (b d e) -> p (u b) d e", d=4, e=4)
                r4 = rep_ap.rearrange("p u (b c d) -> p (u b) c d", c=4, d=4)
                tmp = rotp.tile([128, TS, DIM], f16, tag=dest_tag + "_tmp")
                d4 = dest[:].rearrange("p u (b c e) -> p (u b) c e", c=4, e=4)
                t4 = tmp[:].rearrange("p u (b c e) -> p (u b) c e", c=4, e=4)
                for d in range(4):
                    if transposed:
                        # out[ub,c,e] += rep[ub,d,c] * x[ub,d,e]
                        a_d = r4[:, :, d, :].unsqueeze(3)
                    else:
                        # out[ub,c,e] += rep[ub,c,d] * x[ub,d,e]
                        a_d = r4[:, :, :, d].unsqueeze(3)
                    a_d = a_d.broadcast_to((128, UB, 4, 4))
                    b_d = x4[:, :, d, :].unsqueeze(2).broadcast_to(
                        (128, UB, 4, 4))
                    nc.vector.tensor_tensor(d4 if d == 0 else t4, a_d, b_d,
                                            op=mybir.AluOpType.mult)
                    if d > 0:
                        nc.vector.tensor_tensor(dest[:], dest[:], tmp[:],
                                                op=mybir.AluOpType.add)

            def sh_rows(dram, s):
                """[128, TS, DIM] view of a shard's super-tile s rows."""
                return dram[s * TS * 128:(s + 1) * TS * 128, :].rearrange(
                    "(q p) d -> p q d", p=128)

            # ---- phase R1: g0 = rotate(x * a)   (a pre-folded into x)
            def phase_r1(g0_sh):
              for s in range(NS):
                xp = iop.tile([128, TS, DIM], f16, tag="xp")
                nc.sync.dma_start(xp[:], sh_rows(x_sh, s))
                rp = iop.tile([128, TS, DIM], f16, tag="rp")
                nc.sync.dma_start(rp[:], sh_rows(rep_sh, s))
                g0p = op.tile([128, TS, DIM], f16, tag="g0p")
                rotation_st(xp[:], rp[:], g0p, "g0p", transposed=False)
                nc.sync.dma_start(sh_rows(g0_sh, s), g0p[:])

            def onehots_tk(t, k, tag):
                """Batched one-hot build for bucket (t, k): one DVE
                tensor_tensor is_equal over [128, G, 128]."""
                gcnt = int(Gmat[t][k])
                m2 = m2p.tile([128, g_tk_max, 128], f16, tag=tag)
                g0c = int(g_start[t][k])
                if "onehot" in ablate:
                    nc.vector.tensor_scalar(m2[:, 0, 0:8], iota_sb[:, 0:8],
                                            0.0, None,
                                            op0=mybir.AluOpType.mult)
                elif onehot_mode == "ts":
                    for j in range(gcnt):
                        nc.vector.tensor_scalar(
                            m2[:, j, :], iota_sb[:],
                            dst32_sb[:, g0c + j:g0c + j + 1], None,
                            op0=mybir.AluOpType.is_equal)
                else:
                    i_b = iota_sb[:].unsqueeze(1).broadcast_to(
                        (128, gcnt, 128))
                    d_b = dst_sb[:, g0c:g0c + gcnt].unsqueeze(2).broadcast_to(
                        (128, gcnt, 128))
                    nc.vector.tensor_tensor(m2[:, 0:gcnt, :], i_b, d_b,
                                            op=mybir.AluOpType.is_equal)
                return m2

            def prop_step(g_rep, alloc_cb, emit_cb, flush_cb):
                """One propagation step. One merged gather per (super-tile,
                chunk); one-hots either streamed from HBM (host_fp8) or built
                on DVE; matmuls accumulate in PSUM. Outputs are batched per
                super-tile via the callbacks."""
                for s in range(NS):
                    gths = {}
                    m2st = None
                    gs0 = int(g_start[s * TS][0])
                    if onehot_mode == "host_fp8":
                        # one ACT-queue DMA covers the super-tile's whole
                        # contiguous group range (all 4 chunks)
                        gw = int(gst[s])
                        m2st = m2p.tile([128, gst_max, 128], f8, tag="m2st")
                        if "onehot" not in ablate:
                            eng = nc.scalar if s % 2 else nc.sync
                            eng.dma_start(m2st[:, 0:gw, :],
                                          m2_all[:, gs0:gs0 + gw, :])
                    for k in range(N_CHUNKS):
                        gc = int(gsk[s][k])
                        if gc == 0:
                            continue
                        c0 = int(g_start[s * TS][k])
                        gth = gp.tile([128, gsk_max, DIM], f16, tag="gth")
                        if "gather" not in ablate:
                            nc.gpsimd.dma_gather(
                                gth[:, 0:gc, :],
                                g_rep[k * CHUNK:(k + 1) * CHUNK, :],
                                idx_sb[:, c0 * 8:(c0 + gc) * 8],
                                128 * gc, 128 * gc, DIM,
                                single_packet=False,
                                queue_num=(s * N_CHUNKS + k) % n_queues,
                            )
                        else:  # timing placeholder: cheap small copy
                            nc.sync.dma_start(gth[:, 0, :], g_rep[0:128, :])
                        gths[k] = gth
                    bt = alloc_cb(s)
                    for u in range(TS):
                        t = s * TS + u
                        acc = pp.tile([128, DIM], f32, tag="acc")
                        n_mm = int(Gmat[t].sum())
                        mm = 0
                        for k in range(N_CHUNKS):
                            gcnt = int(Gmat[t][k])
                            if gcnt == 0:
                                continue
                            woff = int(g_start[t][k] - g_start[s * TS][k])
                            if onehot_mode == "host_fp8":
                                m2t, off = m2st, int(g_start[t][k]) - gs0
                            else:
                                m2t, off = onehots_tk(t, k, "m2"), 0
                            for j in range(gcnt):
                                if "mm" not in ablate or mm == 0:
                                    nc.tensor.matmul(
                                        acc[:], m2t[:, off + j, :],
                                        gths[k][:, woff + j, :],
                                        start=(mm == 0),
                                        stop=(mm == n_mm - 1),
                                    )
                                mm += 1
                        emit_cb(t, u, acc, bt)
                    flush_cb(s, bt)

            # ---- phase P2 + inverse rotation + FFN
            # emit per tile: h2 (scaled) parked in a super-tile buffer; after
            # the super-tile completes: batched inverse rotation, then per-tile
            # transpose + FFN.
            st_ctx = {}

            def p2_alloc(s):
                rp2 = iop.tile([128, TS, DIM], f16, tag="rp2")
                nc.sync.dma_start(rp2[:], sh_rows(rep_sh, s))
                h2b = rotp.tile([128, TS, DIM], f16, tag="h2b", name="h2b")
                st_ctx[s] = (rp2, h2b)
                return op.tile([128, TS, DIM], f32, tag="op2", name="op2")

            def p2_emit(t, u, acc, bt):
                s = t // TS
                _, h2b = st_ctx[s]
                nc.scalar.mul(h2b[:, u, :], acc[:], a_sb[:, t:t + 1])

            def p2_flush(s, bt):
                rp2, h2b = st_ctx.pop(s)
                h3b = rotp.tile([128, TS, DIM], f16, tag="h3b")
                rotation_st(h2b[:], rp2[:], h3b, "h3b", transposed=True)
                for u in range(TS):
                    # transpose to [feat, node]
                    tp = pp.tile([128, 128], f32, tag="tp")
                    h3f = rotp.tile([128, 128], f32, tag="h3f")
                    nc.scalar.copy(h3f[:], h3b[:, u, :])
                    nc.tensor.transpose(tp[:], h3f[:], id_sb[:])
                    h3t = rotp.tile([128, 128], bf16, tag="h3t")
                    nc.scalar.copy(h3t[:], tp[:])
                    # FFN layer 1 + exact GELU
                    act = rotp.tile([128, 2, 128], bf16, tag="act")
                    for h in range(2):
                        ps1 = pp.tile([128, 128], f32, tag="ps1")
                        nc.tensor.matmul(ps1[:],
                                         w1t_sb[:, h * 128:(h + 1) * 128],
                                         h3t[:], start=True, stop=True)
                        nc.scalar.activation(
                            act[:, h, :], ps1[:],
                            mybir.ActivationFunctionType.Gelu,
                            bias=b1h_sb[:, h:h + 1])
                    # FFN layer 2 + bias
                    ps2 = pp.tile([128, 128], f32, tag="ps2")
                    for h in range(2):
                        nc.tensor.matmul(ps2[:], w2t2_sb[:, h, :],
                                         act[:, h, :],
                                         start=(h == 0), stop=(h == 1))
                    nc.scalar.activation(bt[:, u, :], ps2[:],
                                         mybir.ActivationFunctionType.Identity,
                                         bias=b2c_sb[:])
                nc.sync.dma_start(
                    out_t[:, s * TS * 128:(s + 1) * TS * 128],
                    bt[:].rearrange("p q d -> p (q d)"))

            for pr in range(pipeline_reps):
                g0_sh = dp.tile([NSHP, DIM], f16, tag=f"g0_sh{pr}",
                                name=f"g0_sh{pr}")
                g0_rep = dp.tile([NREP, DIM], f16, addr_space=rep_space,
                                 tag=f"g0_rep{pr}", name=f"g0_rep{pr}")
                g1_sh = dp.tile([NSHP, DIM], f16, tag=f"g1_sh{pr}",
                                name=f"g1_sh{pr}")
                g1_rep = dp.tile([NREP, DIM], f16, addr_space=rep_space,
                                 tag=f"g1_rep{pr}", name=f"g1_rep{pr}")
                phase_r1(g0_sh)
                allgather(g0_sh, g0_rep)
                # ---- phase P1: g1 = A^T g0, scaled by a^2
                prop_step(
                    g0_rep,
                    lambda s: op.tile([128, TS, DIM], f16, tag="g1p",
                                      name="g1p"),
                    lambda t, u, acc, bt: nc.scalar.mul(
                        bt[:, u, :], acc[:], a2_sb[:, t:t + 1]),
                    lambda s, bt, _g=g1_sh: nc.sync.dma_start(
                        sh_rows(_g, s), bt[:]),
                )
                allgather(g1_sh, g1_rep)
                prop_step(g1_rep, p2_alloc, p2_emit, p2_flush)

    nc.compile()
    return nc


# -------------------------------------------------------------------- runner

def kernel(x, node_rep, src, dst, w1, b1, w2, b2):
    global LAST_RESULTS, LAST_NC, LAST_IN_MAPS
    from concourse import bass_utils

    x = np.asarray(x, np.float32)
    node_rep = np.asarray(node_rep, np.float32)
    src = np.asarray(src, np.int64)
    dst = np.asarray(dst, np.int64)
    w1 = np.asarray(w1, np.float32)
    b1 = np.asarray(b1, np.float32)
    w2 = np.asarray(w2, np.float32)
    b2 = np.asarray(b2, np.float32)

    in_maps, structure, total_groups = preprocess(
        x, node_rep, src, dst, w1, b1, w2, b2)
    nc = build_nc(structure, total_groups)
    res = bass_utils.run_bass_kernel_spmd(
        nc, in_maps, core_ids=list(range(N_CORES)),
        trace=bool(os.environ.get("BASS_TRACE")),
    )
    LAST_RESULTS = res
    LAST_NC = nc
    LAST_IN_MAPS = in_maps
    out = np.concatenate(
        [res.results[c]["out_t"].T[:NSH] for c in range(N_CORES)], axis=0)
    return np.ascontiguousarray(out)


# revision 39
# speedup vs baseline: 1.5927x; 1.5927x over previous
"""Trainium2 Bass kernel for GNN message passing (nn_BDLModule_34488587387542).

Computation (N=100000 nodes, E=1600000 edges, DIM=128):
    deg  = out-degree(src);  a = rsqrt(deg)
    h0   = per-node block rotation of x (8 bundles of 4x4)
    h2   = S S h0,  S = diag(a) A^T diag(a)   (2 propagation steps)
    h3   = inverse rotation of h2
    out  = GELU_exact(h3 @ w1.T + b1) @ w2.T + b2

Sharding: nodes partitioned contiguously across 8 cores (12500 each). Edges
bucketed by owning dst shard; per core, grouped by (dst tile of 128, src
chunk of 25088 replica rows) so gathers use int16 indices, and sorted by
src row within each bucket so the dma_gather descriptors walk ascending
HBM addresses (big win: random 256B row reads are row-miss bound). The
propagation step is: dma_gather rows from the replicated node table; the
one-hot dst matrices are PRECOMPUTED ON THE HOST as fp8e4 (exact 0/1) and
streamed from HBM (onehot_mode="host_fp8"; "tt"/"ts" build them on DVE);
PE matmuls (fp8 stationary x fp16 moving) accumulate into PSUM. AllGather
replicates the node table between steps (measured ~8us each - cheap). The
separable norm coefs a[src]*a[dst] are folded into the stored node tables;
the R1-side `a` is folded into x on the host. All rotation math runs in
fp16 on DVE (batched per supertile); FFN matmuls run in bf16.

build_nc(pipeline_reps=N) chains N independent copies of the whole pipeline
in one NEFF (slope timing tool; kernel() uses N=1).
"""
import os
import sys

sys.path.append("/opt/trn_rl_repo")

import numpy as np

N_NODES = 100000
N_EDGES = 1600000
DIM = 128
HID = 256
N_CORES = 8
NSH = 12500                 # nodes per shard
NSHP = 12544                # padded shard rows (98 * 128)
NT = NSHP // 128            # dst tiles per core = 98
NREP = NSHP * N_CORES       # replica table rows = 100352
CHUNK = NREP // 4           # gather chunk rows = 25088 (int16-addressable)
N_CHUNKS = 4
PAD_DST = 1000.0            # dst_local sentinel for padding edges
TS = 7                      # dst tiles per super-tile (gathers merged per
NS = NT // TS               # (super-tile, chunk) to amortize SWDGE overhead)

# module globals: last run state (test.py reuses these for timing)
LAST_RESULTS = None
LAST_NC = None
LAST_IN_MAPS = None


# ----------------------------------------------------------------- host prep

def _wrap_idx(idx_flat: np.ndarray) -> np.ndarray:
    """[n] -> [128, n/16] int16 wrapped+replicated dma_gather index layout."""
    w = idx_flat.reshape(-1, 16).T.astype(np.int16)
    return np.tile(w, (8, 1))


def preprocess(x, node_rep, src, dst, w1, b1, w2, b2):
    """Build per-core input maps + the static SPMD edge-group structure."""
    deg = np.bincount(src, minlength=N_NODES).astype(np.float64)
    a64 = 1.0 / np.sqrt(deg)
    a = a64.astype(np.float32)
    a2 = (1.0 / deg).astype(np.float32)

    # global node id -> replica-table row
    def rrow(u):
        return (u // NSH) * NSHP + (u % NSH)

    src_rrow = rrow(src)
    dst_core = dst // NSH

    # Buckets ordered (super-tile s, chunk k, tile-within-s): one gather per
    # (s, k) covers TS tiles' groups contiguously.
    n_buckets = NT * N_CHUNKS
    counts = np.zeros((N_CORES, n_buckets), np.int64)
    per_core = []
    for c in range(N_CORES):
        m = dst_core == c
        dl = (dst[m] - c * NSH).astype(np.int64)      # local dst
        sr = src_rrow[m]
        tile_id = dl // 128
        chunk_id = sr // CHUNK
        key = ((tile_id // TS) * N_CHUNKS + chunk_id) * TS + tile_id % TS
        # secondary sort by src row: ascending gather addresses per bucket
        order = np.argsort(key * (1 << 18) + sr, kind="stable")
        per_core.append((dl[order], sr[order], key[order]))
        counts[c] = np.bincount(key, minlength=n_buckets)

    G = np.ceil(counts.max(axis=0) / 128.0).astype(np.int64)  # [n_buckets]
    g_off = np.concatenate([[0], np.cumsum(G)])               # group offsets
    total_groups = int(g_off[-1])
    total_edges_padded = total_groups * 128

    # tile-major views for the build loops
    Gmat = np.zeros((NT, N_CHUNKS), np.int64)
    g_start = np.zeros((NT, N_CHUNKS), np.int64)
    for b in range(n_buckets):
        s, k, u = b // (N_CHUNKS * TS), (b // TS) % N_CHUNKS, b % TS
        t = s * TS + u
        Gmat[t, k] = G[b]
        g_start[t, k] = g_off[b]
    structure = {"Gmat": Gmat, "g_start": g_start}

    iota = np.tile(np.arange(128, dtype=np.float16), (128, 1))
    ident = np.eye(128, dtype=np.float32)
    w1t = np.ascontiguousarray(w1.T.astype(np.float32))        # [DIM, HID]
    b1h = np.ascontiguousarray(b1.reshape(2, 128).T)           # [128, 2]
    w2t2 = np.ascontiguousarray(
        w2.T.reshape(2, 128, DIM).transpose(1, 0, 2).astype(np.float32))
    b2c = np.ascontiguousarray(b2.reshape(128, 1))

    in_maps = []
    for c in range(N_CORES):
        dl, sr, key = per_core[c]
        n = dl.shape[0]
        # position of each edge inside the padded stream
        bucket_starts = g_off[:-1] * 128                    # [n_buckets]
        within = np.arange(n) - np.concatenate(
            [[0], np.cumsum(np.bincount(key, minlength=n_buckets))]
        )[key]
        pos = bucket_starts[key] + within
        idx_pad = np.zeros(total_edges_padded, np.int64)
        dst_pad = np.full(total_edges_padded, PAD_DST, np.float16)
        idx_pad[pos] = sr % CHUNK
        dst_pad[pos] = (dl % 128).astype(np.float16)

        rows = slice(c * NSH, (c + 1) * NSH)
        # R1-side `a` folded into x here (rotation is linear per node)
        x_sh = np.zeros((NSHP, DIM), np.float16)
        x_sh[:NSH] = (x[rows] * a[rows, None]).astype(np.float16)
        rep_sh = np.zeros((NSHP, DIM), np.float16)
        rep_sh[:NSH] = node_rep[rows].reshape(NSH, DIM).astype(np.float16)
        a_pad = np.zeros(NSHP, np.float32)
        a_pad[:NSH] = a[rows]
        a2_pad = np.zeros(NSHP, np.float32)
        a2_pad[:NSH] = a2[rows]

        # host-built one-hot matrices, fp8e4 (exact 0/1): [128, tg, 128]
        # m2_host[p, g, f] = 1 iff edge slot p of group g has dst row f
        # (0x38 is the e4m3 bit pattern of 1.0 - avoids slow ml_dtypes cast)
        from ml_dtypes import float8_e4m3
        dst_g = dst_pad.reshape(total_groups, 128).astype(np.int32)  # [g, p]
        m2_host = np.zeros((128, total_groups, 128), np.uint8)
        gi, pi = np.nonzero(dst_g < 128)
        m2_host[pi, gi, dst_g[gi, pi]] = 0x38
        m2_host = m2_host.view(float8_e4m3)

        in_maps.append({
            "x_sh": x_sh,
            "rep_sh": rep_sh,
            "idx_all": _wrap_idx(idx_pad),                       # [128, tg*8]
            "dst_all": np.ascontiguousarray(
                dst_pad.reshape(total_groups, 128).T),           # [128, tg] f16
            "m2_all": m2_host,                                   # [128, tg, 128]
            "a_col": np.ascontiguousarray(
                a_pad.reshape(NT, 128).T),                       # [128, NT]
            "a2_col": np.ascontiguousarray(
                a2_pad.reshape(NT, 128).T),                      # [128, NT]
            "iota": iota,
            "ident": ident,
            "w1t": w1t,
            "b1h": b1h,
            "w2t2": w2t2,
            "b2c": b2c,
        })
    return in_maps, structure, total_groups


# -------------------------------------------------------------- device build

def build_nc(structure, total_groups, single_core_timing=False, ablate=(),
             n_queues=4, onehot_mode="host_fp8", pipeline_reps=1):
    import concourse.bacc as bacc
    import concourse.mybir as mybir
    import concourse.tile as tile

    f32 = mybir.dt.float32
    f16 = mybir.dt.float16
    bf16 = mybir.dt.bfloat16
    f8 = mybir.dt.float8e4
    nc = bacc.Bacc("TRN2", target_bir_lowering=False, debug=False,
                   num_devices=1 if single_core_timing else N_CORES,
                   num_swdge_queues=n_queues)

    x_sh = nc.dram_tensor("x_sh", [NSHP, DIM], f16, kind="ExternalInput")
    rep_sh = nc.dram_tensor("rep_sh", [NSHP, DIM], f16, kind="ExternalInput")
    idx_all = nc.dram_tensor("idx_all", [128, total_groups * 8],
                             mybir.dt.int16, kind="ExternalInput")
    dst_all = nc.dram_tensor("dst_all", [128, total_groups], f16,
                             kind="ExternalInput")
    m2_all = nc.dram_tensor("m2_all", [128, total_groups, 128], f8,
                            kind="ExternalInput")
    a_col = nc.dram_tensor("a_col", [128, NT], f32, kind="ExternalInput")
    a2_col = nc.dram_tensor("a2_col", [128, NT], f32, kind="ExternalInput")
    iota = nc.dram_tensor("iota", [128, 128], f16, kind="ExternalInput")
    ident = nc.dram_tensor("ident", [128, 128], f32, kind="ExternalInput")
    w1t = nc.dram_tensor("w1t", [DIM, HID], f32, kind="ExternalInput")
    b1h = nc.dram_tensor("b1h", [128, 2], f32, kind="ExternalInput")
    w2t2 = nc.dram_tensor("w2t2", [128, 2, DIM], f32, kind="ExternalInput")
    b2c = nc.dram_tensor("b2c", [128, 1], f32, kind="ExternalInput")
    out_t = nc.dram_tensor("out_t", [DIM, NSHP], f32, kind="ExternalOutput")

    Gmat = structure["Gmat"]
    g_start = structure["g_start"]
    # widest merged gather (groups) over (super-tile, chunk)
    gsk = Gmat.reshape(NS, TS, N_CHUNKS).sum(axis=1)        # [NS, N_CHUNKS]
    gsk_max = int(gsk.max())
    # widest one-hot batch (groups) over (tile, chunk) buckets
    g_tk_max = int(Gmat.max())
    # widest whole-super-tile group range (all 4 chunks, contiguous)
    gst = gsk.sum(axis=1)                                   # [NS]
    gst_max = int(gst.max())

    with tile.TileContext(nc) as tc:
        with (
            tc.tile_pool(name="const", bufs=1) as cp,
            tc.tile_pool(name="io", bufs=2) as iop,
            tc.tile_pool(name="rotp", bufs=2) as rotp,
            tc.tile_pool(name="gath", bufs=8) as gp,
            tc.tile_pool(name="m2", bufs=2 if onehot_mode == "host_fp8"
                         else 8) as m2p,
            tc.tile_pool(name="outp", bufs=2) as op,
            tc.tile_pool(name="psum", bufs=2, space="PSUM") as pp,
            tc.tile_pool(name="dram", bufs=1, space="DRAM") as dp,
        ):
            # ---- constants into SBUF
            iota_sb = cp.tile([128, 128], f16)
            nc.sync.dma_start(iota_sb[:], iota[:])
            id_sb = cp.tile([128, 128], f32)
            nc.sync.dma_start(id_sb[:], ident[:])
            idx_sb = cp.tile([128, total_groups * 8], mybir.dt.int16)
            nc.sync.dma_start(idx_sb[:], idx_all[:])
            if onehot_mode != "host_fp8":
                dst_sb = cp.tile([128, total_groups], f16)
                nc.sync.dma_start(dst_sb[:], dst_all[:])
            if onehot_mode == "ts":
                dst32_sb = cp.tile([128, total_groups], f32)
                nc.vector.tensor_copy(dst32_sb[:], dst_sb[:])
            a_sb = cp.tile([128, NT], f32)
            nc.sync.dma_start(a_sb[:], a_col[:])
            a2_sb = cp.tile([128, NT], f32)
            nc.sync.dma_start(a2_sb[:], a2_col[:])
            w1t_sb = cp.tile([DIM, HID], bf16)
            w1t_f32 = cp.tile([DIM, HID], f32)
            nc.sync.dma_start(w1t_f32[:], w1t[:])
            nc.vector.tensor_copy(w1t_sb[:], w1t_f32[:])
            b1h_sb = cp.tile([128, 2], f32)
            nc.sync.dma_start(b1h_sb[:], b1h[:])
            w2t2_sb = cp.tile([128, 2, DIM], bf16)
            w2t2_f32 = cp.tile([128, 2, DIM], f32)
            nc.sync.dma_start(w2t2_f32[:], w2t2[:])
            nc.vector.tensor_copy(w2t2_sb[:], w2t2_f32[:])
            b2c_sb = cp.tile([128, 1], f32)
            nc.sync.dma_start(b2c_sb[:], b2c[:])

            rep_space = "Local" if single_core_timing else "Shared"

            def allgather(sh, rep):
                if single_core_timing:
                    # timing stand-in: local copy keeps the data dependency
                    nc.sync.dma_start(rep[0:NSHP, :], sh[:])
                else:
                    nc.gpsimd.collective_compute(
                        "AllGather", mybir.AluOpType.bypass,
                        ins=[sh.opt()], outs=[rep.opt()],
                        replica_groups=[list(range(N_CORES))],
                    )

            def rotation_st(x_ap, rep_ap, dest, dest_tag, transposed):
                """Whole-super-tile rotation: dest = einsum(rep, x) per node.

                x_ap/rep_ap/dest: [128, TS, DIM] fp16. The (u, b) dims merge
                (stride(u)=128=8*16=8*stride(b)) so free rank stays <= 3.
                """
                UB = TS * 8
                x4 = x_ap.rearrange("p u (b d e) -> p (u b) d e", d=4, e=4)
                r4 = rep_ap.rearrange("p u (b c d) -> p (u b) c d", c=4, d=4)
                tmp = rotp.tile([128, TS, DIM], f16, tag=dest_tag + "_tmp")
                d4 = dest[:].rearrange("p u (b c e) -> p (u b) c e", c=4, e=4)
                t4 = tmp[:].rearrange("p u (b c e) -> p (u b) c e", c=4, e=4)
                for d in range(4):
                    if transposed:
                        # out[ub,c,e] += rep[ub,d,c] * x[ub,d,e]
                        a_d = r4[:, :, d, :].unsqueeze(3)
                    else:
                        # out[ub,c,e] += rep[ub,c,d] * x[ub,d,e]
                        a_d = r4[:, :, :, d].unsqueeze(3)
                    a_d = a_d.broadcast_to((128, UB, 4, 4))
                    b_d = x4[:, :, d, :].unsqueeze(2).broadcast_to(
                        (128, UB, 4, 4))
                    nc.vector.tensor_tensor(d4 if d == 0 else t4, a_d, b_d,
                                            op=mybir.AluOpType.mult)
                    if d > 0:
                        nc.vector.tensor_tensor(dest[:], dest[:], tmp[:],
                                                op=mybir.AluOpType.add)

            def sh_rows(dram, s):
                """[128, TS, DIM] view of a shard's super-tile s rows."""
                return dram[s * TS * 128:(s + 1) * TS * 128, :].rearrange(
                    "(q p) d -> p q d", p=128)

            # ---- phase R1: g0 = rotate(x * a)   (a pre-folded into x)
            def phase_r1(g0_sh):
              for s in range(NS):
                xp = iop.tile([128, TS, DIM], f16, tag="xp")
                nc.sync.dma_start(xp[:], sh_rows(x_sh, s))
                rp = iop.tile([128, TS, DIM], f16, tag="rp")
                nc.sync.dma_start(rp[:], sh_rows(rep_sh, s))
                g0p = op.tile([128, TS, DIM], f16, tag="g0p")
                rotation_st(xp[:], rp[:], g0p, "g0p", transposed=False)
                nc.sync.dma_start(sh_rows(g0_sh, s), g0p[:])

            def onehots_tk(t, k, tag):
                """Batched one-hot build for bucket (t, k): one DVE
                tensor_tensor is_equal over [128, G, 128]."""
                gcnt = int(Gmat[t][k])
                m2 = m2p.tile([128, g_tk_max, 128], f16, tag=tag)
                g0c = int(g_start[t][k])
                if "onehot" in ablate:
                    nc.vector.tensor_scalar(m2[:, 0, 0:8], iota_sb[:, 0:8],
                                            0.0, None,
                                            op0=mybir.AluOpType.mult)
                elif onehot_mode == "ts":
                    for j in range(gcnt):
                        nc.vector.tensor_scalar(
                            m2[:, j, :], iota_sb[:],
                            dst32_sb[:, g0c + j:g0c + j + 1], None,
                            op0=mybir.AluOpType.is_equal)
                else:
                    i_b = iota_sb[:].unsqueeze(1).broadcast_to(
                        (128, gcnt, 128))
                    d_b = dst_sb[:, g0c:g0c + gcnt].unsqueeze(2).broadcast_to(
                        (128, gcnt, 128))
                    nc.vector.tensor_tensor(m2[:, 0:gcnt, :], i_b, d_b,
                                            op=mybir.AluOpType.is_equal)
                return m2

            def prop_step(g_rep, alloc_cb, emit_cb, flush_cb):
                """One propagation step. One merged gather per (super-tile,
                chunk); one-hots either streamed from HBM (host_fp8) or built
                on DVE; matmuls accumulate in PSUM. Outputs are batched per
                super-tile via the callbacks."""
                for s in range(NS):
                    gths = {}
                    m2st = None
                    gs0 = int(g_start[s * TS][0])
                    if onehot_mode == "host_fp8":
                        # one ACT-queue DMA covers the super-tile's whole
                        # contiguous group range (all 4 chunks)
                        gw = int(gst[s])
                        m2st = m2p.tile([128, gst_max, 128], f8, tag="m2st")
                        if "onehot" not in ablate:
                            eng = nc.scalar if s % 2 else nc.sync
                            eng.dma_start(m2st[:, 0:gw, :],
                                          m2_all[:, gs0:gs0 + gw, :])
                    for k in range(N_CHUNKS):
                        gc = int(gsk[s][k])
                        if gc == 0:
                            continue
                        c0 = int(g_start[s * TS][k])
                        gth = gp.tile([128, gsk_max, DIM], f16, tag="gth")
                        if "gather" not in ablate:
                            nc.gpsimd.dma_gather(
                                gth[:, 0:gc, :],
                                g_rep[k * CHUNK:(k + 1) * CHUNK, :],
                                idx_sb[:, c0 * 8:(c0 + gc) * 8],
                                128 * gc, 128 * gc, DIM,
                                single_packet=False,
                                queue_num=k % n_queues,
                            )
                        else:  # timing placeholder: cheap small copy
                            nc.sync.dma_start(gth[:, 0, :], g_rep[0:128, :])
                        gths[k] = gth
                    bt = alloc_cb(s)
                    for u in range(TS):
                        t = s * TS + u
                        acc = pp.tile([128, DIM], f32, tag="acc")
                        n_mm = int(Gmat[t].sum())
                        mm = 0
                        for k in range(N_CHUNKS):
                            gcnt = int(Gmat[t][k])
                            if gcnt == 0:
                                continue
                            woff = int(g_start[t][k] - g_start[s * TS][k])
                            if onehot_mode == "host_fp8":
                                m2t, off = m2st, int(g_start[t][k]) - gs0
                            else:
                                m2t, off = onehots_tk(t, k, "m2"), 0
                            for j in range(gcnt):
                                if "mm" not in ablate or mm == 0:
                                    nc.tensor.matmul(
                                        acc[:], m2t[:, off + j, :],
                                        gths[k][:, woff + j, :],
                                        start=(mm == 0),
                                        stop=(mm == n_mm - 1),
                                    )
                                mm += 1
                        emit_cb(t, u, acc, bt)
                    flush_cb(s, bt)

            # ---- phase P2 + inverse rotation + FFN
            # emit per tile: h2 (scaled) parked in a super-tile buffer; after
            # the super-tile completes: batched inverse rotation, then per-tile
            # transpose + FFN.
            st_ctx = {}

            def p2_alloc(s):
                rp2 = iop.tile([128, TS, DIM], f16, tag="rp2")
                nc.sync.dma_start(rp2[:], sh_rows(rep_sh, s))
                h2b = rotp.tile([128, TS, DIM], f16, tag="h2b", name="h2b")
                st_ctx[s] = (rp2, h2b)
                return op.tile([128, TS, DIM], f32, tag="op2", name="op2")

            def p2_emit(t, u, acc, bt):
                s = t // TS
                _, h2b = st_ctx[s]
                nc.scalar.mul(h2b[:, u, :], acc[:], a_sb[:, t:t + 1])

            def p2_flush(s, bt):
                rp2, h2b = st_ctx.pop(s)
                h3b = rotp.tile([128, TS, DIM], f16, tag="h3b")
                rotation_st(h2b[:], rp2[:], h3b, "h3b", transposed=True)
                for u in range(TS):
                    # transpose to [feat, node]
                    tp = pp.tile([128, 128], f32, tag="tp")
                    h3f = rotp.tile([128, 128], f32, tag="h3f")
                    nc.scalar.copy(h3f[:], h3b[:, u, :])
                    nc.tensor.transpose(tp[:], h3f[:], id_sb[:])
                    h3t = rotp.tile([128, 128], bf16, tag="h3t")
                    nc.scalar.copy(h3t[:], tp[:])
                    # FFN layer 1 + exact GELU
                    act = rotp.tile([128, 2, 128], bf16, tag="act")
                    for h in range(2):
                        ps1 = pp.tile([128, 128], f32, tag="ps1")
                        nc.tensor.matmul(ps1[:],
                                         w1t_sb[:, h * 128:(h + 1) * 128],
                                         h3t[:], start=True, stop=True)
                        nc.scalar.activation(
                            act[:, h, :], ps1[:],
                            mybir.ActivationFunctionType.Gelu,
                            bias=b1h_sb[:, h:h + 1])
                    # FFN layer 2 + bias
                    ps2 = pp.tile([128, 128], f32, tag="ps2")
                    for h in range(2):
                        nc.tensor.matmul(ps2[:], w2t2_sb[:, h, :],
                                         act[:, h, :],
                                         start=(h == 0), stop=(h == 1))
                    nc.scalar.activation(bt[:, u, :], ps2[:],
                                         mybir.ActivationFunctionType.Identity,
                                         bias=b2c_sb[:])
                nc.sync.dma_start(
                    out_t[:, s * TS * 128:(s + 1) * TS * 128],
                    bt[:].rearrange("p q d -> p (q d)"))

            for pr in range(pipeline_reps):
                g0_sh = dp.tile([NSHP, DIM], f16, tag=f"g0_sh{pr}",
                                name=f"g0_sh{pr}")
                g0_rep = dp.tile([NREP, DIM], f16, addr_space=rep_space,
                                 tag=f"g0_rep{pr}", name=f"g0_rep{pr}")
                g1_sh = dp.tile([NSHP, DIM], f16, tag=f"g1_sh{pr}",
                                name=f"g1_sh{pr}")
                g1_rep = dp.tile([NREP, DIM], f16, addr_space=rep_space,
                                 tag=f"g1_rep{pr}", name=f"g1_rep{pr}")
                phase_r1(g0_sh)
                allgather(g0_sh, g0_rep)
                # ---- phase P1: g1 = A^T g0, scaled by a^2
                prop_step(
                    g0_rep,
                    lambda s: op.tile([128, TS, DIM], f16, tag="g1p",
                                      name="g1p"),
                    lambda t, u, acc, bt: nc.scalar.mul(
                        bt[:, u, :], acc[:], a2_sb[:, t:t + 1]),
                    lambda s, bt, _g=g1_sh: nc.sync.dma_start(
                        sh_rows(_g, s), bt[:]),
                )
                allgather(g1_sh, g1_rep)
                prop_step(g1_rep, p2_alloc, p2_emit, p2_flush)

    nc.compile()
    return nc


# -------------------------------------------------------------------- runner

def kernel(x, node_rep, src, dst, w1, b1, w2, b2):
    global LAST_RESULTS, LAST_NC, LAST_IN_MAPS
    from concourse import bass_utils

    x = np.asarray(x, np.float32)
    node_rep = np.asarray(node_rep, np.float32)
    src = np.asarray(src, np.int64)
    dst = np.asarray(dst, np.int64)
    w1 = np.asarray(w1, np.float32)
    b1 = np.asarray(b1, np.float32)
    w2 = np.asarray(w2, np.float32)
    b2 = np.asarray(b2, np.float32)

    in_maps, structure, total_groups = preprocess(
        x, node_rep, src, dst, w1, b1, w2, b2)
    nc = build_nc(structure, total_groups)
    res = bass_utils.run_bass_kernel_spmd(
        nc, in_maps, core_ids=list(range(N_CORES)),
        trace=bool(os.environ.get("BASS_TRACE")),
    )
    LAST_RESULTS = res
    LAST_NC = nc
    LAST_IN_MAPS = in_maps
    out = np.concatenate(
        [res.results[c]["out_t"].T[:NSH] for c in range(N_CORES)], axis=0)
    return np.ascontiguousarray(out)
